# revision 22
# baseline (speedup 1.0000x reference)
"""Trainium2 Bass kernel: HMM forward algorithm (log-space) for AugmentedModel.log_prob.

Probability-domain forward recurrence with lagged periodic rescaling:
    w   = alpha ⊙ q_t              (q_t = exp(emission+policy log-prob row))
    u   = w @ P[a_t[b]]            (per-batch transition matrix, 8 choices)
    alpha ∝ u                      (normalize every NORM_EVERY steps, lagged)
log_prob[b] = sum of logs of the removed scales (telescopes exactly).

Per-batch transition selection by zero-masked matmul accumulation over the 8
actions (masks are disjoint one-hots over actions so the PSUM sum IS the
selection); 4 PE column groups run 2 actions each concurrently, stationaries
zero-padded to the full 32-column group width.

Per-step pipeline (critical path = chain -> copy1 -> transpose kc01 ->
reduce kc01 -> w8 kc01 -> next chain; kc23 and all event math overlap):
  PE:     8 rounds of 4 concurrent N=512 masked MMs   (bf16, PSUM f32)
  Scalar: strips->cs copies, f32->bf16                (carrying the lagged
          1/ell rescale; on event steps accum_out row-sums)
  PE:     one [128,128] bf16 transpose per kc         (cs -> uTT, PSUM)
  DVE:    strided reduce of the 4 column-group strips -> uT (bf16), then
          w8(t+1) = uT ⊙ qm8[t+1]                     (qm8 = exp(lq)*mask
          from HOST, bf16; step-0 alpha0 folded in on host)
Norm events (every 4 steps) are fully off the critical path: copy accum_outs
-> prow -> I4 matmul -> ell[16] (PSUM), reciprocal first, then an I4-style
replicate matmul -> srec[128,1] used by the copies two steps later; frexp
ln(ell) accumulates into logacc.

Sharding: data-parallel over batch B=128 -> 16 episodes per core, tables
replicated; no collectives.
"""

import numpy as np
from contextlib import ExitStack

T, B, S, A, NO, NR = 128, 128, 512, 8, 64, 16
TT = T + 1
NCORES = 8
BC = B // NCORES          # 16 episodes per core
KC = 4                    # 512 states = 4 chunks of 128 partitions
NORM_EVERY = 4
WSL = 32                  # stationary slot width (16 episodes + 16 zero pad)
PREF = 4                  # qm8 DMA prefetch depth


def _host_q_mask(regime, seq_o, seq_r, seq_d, seq_a,
                 log_emit_o, log_emit_r, log_emit_d, log_policy):
    """q[t,b,s] = exp(emission+policy log-prob), msk[t,a,b] one-hot actions."""
    d_all = np.concatenate([seq_d, np.ones((1, B), np.int32)], 0)        # [TT,B]
    d_cum = np.maximum.accumulate(d_all, 0)                              # [TT,B]
    was_d = np.concatenate([np.zeros((1, B), np.int32), d_cum[:-1]], 0)  # [TT,B]
    a_full = np.concatenate([seq_a, np.zeros((1, B), np.int32)], 0)      # [TT,B]

    lq = log_emit_o[seq_o] + log_emit_r[seq_r] + log_emit_d[d_cum]       # [TT,B,S]
    lq = np.where((was_d == 1)[..., None], 0.0, lq)
    lq_a = log_policy[a_full]
    lq_a = np.where((d_cum == 1)[..., None], 0.0, lq_a)
    lq_a = np.where((regime == 1)[None, :, None], 0.0, lq_a)
    q = np.exp((lq + lq_a).astype(np.float32)).astype(np.float32)        # [TT,B,S]
    msk = (a_full[:, None, :] == np.arange(A)[None, :, None]).astype(np.float32)
    return q, msk


def _build_nc():
    import concourse.bass as bass
    import concourse.bacc as bacc
    import concourse.mybir as mybir
    import concourse.tile as tile

    f32 = mybir.dt.float32
    bf16 = mybir.dt.bfloat16
    i32 = mybir.dt.int32
    LN = mybir.ActivationFunctionType.Ln
    CPY = mybir.ActivationFunctionType.Copy
    MUL = mybir.AluOpType.mult
    ADD = mybir.AluOpType.add
    SHR = mybir.AluOpType.logical_shift_right
    BAND = mybir.AluOpType.bitwise_and
    BOR = mybir.AluOpType.bitwise_or
    AX = mybir.AxisListType.X

    nc = bacc.Bacc(None, target_bir_lowering=False)

    qm8_d = nc.dram_tensor("qm8", [TT, 128, A, KC, BC], bf16,
                           kind="ExternalInput")
    pt_d = nc.dram_tensor("ptab", [128, KC, A, S], bf16, kind="ExternalInput")
    idB_d = nc.dram_tensor("identB", [128, 128], bf16, kind="ExternalInput")
    i4_d = nc.dram_tensor("i4t", [128, BC], bf16, kind="ExternalInput")
    i4r_d = nc.dram_tensor("i4rep", [BC, 128], bf16, kind="ExternalInput")
    out_d = nc.dram_tensor("out", [BC, 1], f32, kind="ExternalOutput")

    with tile.TileContext(nc) as tc, ExitStack() as ctx:
        const = ctx.enter_context(tc.tile_pool(name="const", bufs=1))
        qpool = ctx.enter_context(tc.tile_pool(name="qm8", bufs=PREF + 1))
        cpool = ctx.enter_context(tc.tile_pool(name="cs", bufs=2))
        upool = ctx.enter_context(tc.tile_pool(name="uT", bufs=2))
        npool = ctx.enter_context(tc.tile_pool(name="nrm", bufs=2))
        ppA = ctx.enter_context(tc.tile_pool(name="strA", bufs=1, space="PSUM"))
        ppB = ctx.enter_context(tc.tile_pool(name="strB", bufs=1, space="PSUM"))
        tpA = ctx.enter_context(tc.tile_pool(name="uTTA", bufs=1, space="PSUM"))
        tpB = ctx.enter_context(tc.tile_pool(name="uTTB", bufs=1, space="PSUM"))
        epp = ctx.enter_context(tc.tile_pool(name="ellp", bufs=1, space="PSUM"))
        spp = ctx.enter_context(tc.tile_pool(name="srecp", bufs=1, space="PSUM"))

        ptabs = [[const.tile([128, A, S // 2], bf16, name=f"ptab{kc}h{h}")
                  for h in range(2)] for kc in range(KC)]
        identB = const.tile([128, 128], bf16)
        i4t = const.tile([128, BC], bf16)
        i4rep = const.tile([BC, 128], bf16)
        logacc = const.tile([BC, 1], f32)
        srec = const.tile([128, 1], f32)
        nc.vector.memset(logacc[:], 0.0)

        # persistent w8 stationaries; zero pad columns stay zero forever
        w8s = [const.tile([128, A, KC, WSL], bf16, name=f"w8_{i}")
               for i in range(2)]
        for w8 in w8s:
            nc.vector.memset(w8[:], 0.0)

        qm8t = {}

        def fetch(t):
            if t < TT:
                q = qpool.tile([128, A, KC, BC], bf16, tag="qm8",
                               name=f"qm8_{t}")
                nc.sync.dma_start(q[:], qm8_d[t])
                qm8t[t] = q

        for t in range(PREF):
            fetch(t)
        for h in range(2):
            for kc in range(KC):
                nc.sync.dma_start(ptabs[kc][h][:],
                                  pt_d[:, kc, :, h * 256:(h + 1) * 256])
        nc.sync.dma_start(identB[:], idB_d[:])
        nc.sync.dma_start(i4t[:], i4_d[:])
        nc.sync.dma_start(i4rep[:], i4r_d[:])
        nc.vector.tensor_copy(w8s[0][:, :, :, 0:BC], qm8t[0][:])

        def ell_matmul(prow):
            ell = epp.tile([BC, 1], f32, tag="ell")
            nc.tensor.matmul(ell[:], i4t[:], prow[:], start=True, stop=True)
            return ell

        def ln_into_logacc(ell):
            # ln(ell) via frexp: ln(m*2^e) = Ln(m) + (e-127)*ln2
            e_t = npool.tile([BC, 1], i32, tag="e_t")
            nc.vector.tensor_scalar(e_t[:], ell[:].bitcast(i32), 23, None, SHR)
            m_t = npool.tile([BC, 1], i32, tag="m_t")
            nc.vector.tensor_scalar(m_t[:], ell[:].bitcast(i32),
                                    0x007FFFFF, 0x3F800000, BAND, BOR)
            lnb = npool.tile([BC, 1], f32, tag="lnb")
            nc.scalar.activation(lnb[:], m_t[:].bitcast(f32), LN)
            esc = npool.tile([BC, 1], f32, tag="esc")
            nc.scalar.activation(esc[:], e_t[:], CPY,
                                 bias=-88.02969193111305,
                                 scale=0.6931471805599453)
            nc.vector.scalar_tensor_tensor(logacc[:], lnb[:], esc[:, 0:1],
                                           logacc[:], ADD, ADD)

        prow_ev = None        # (prow, t) from an event step, pending ell
        pending_ln = None     # (ell, t): recip + ln at this iteration's tail
        pending_srec = None   # rec16: replicate into srec at next iter top
        for t in range(TT):
            w8 = w8s[t % 2]
            last = t == TT - 1
            is_apply = t > NORM_EVERY and (t - 2) % NORM_EVERY == 0
            is_event = t % NORM_EVERY == 0 and t > 0
            fetch(t + PREF)

            # ---- pending srec replicate (PE matmul + DVE copy) ---------
            # rec16 from the previous iteration's DVE tail is ready by
            # now; srec lands in SBUF well before this step's copies.
            if pending_srec is not None:
                srec_ps = spp.tile([128, 1], f32, tag="srp")
                nc.tensor.matmul(srec_ps[:], i4rep[:], pending_srec[:],
                                 start=True, stop=True)
                nc.vector.tensor_copy(srec[:], srec_ps[:])
                pending_srec = None

            # ---- masked matmul chain: two s'-halves of N=256 MMs -------
            # copies are emitted right after each half so the Scalar copy
            # of half 1 overlaps the half-2 matmuls
            strA = ppA.tile([128, 256], f32, tag="sA")
            strB = ppB.tile([128, 256], f32, tag="sB")
            cs01 = cpool.tile([128, 256], bf16, tag="cs01", name="cs01")
            cs23 = cpool.tile([128, 256], bf16, tag="cs23", name="cs23")
            sc = srec[:, 0:1] if is_apply else 1.0
            aos = None
            if is_event:
                aos = [npool.tile([128, 1], f32, tag=f"ao{i}", name=f"ao{i}")
                       for i in range(2)]
            # (h, kc) block order: delay first kc2 use (w8_23 slack) while
            # strA still completes early enough for copy1 to hide
            halves = ((strA, cs01, 0), (strB, cs23, 1))
            for h, kc in ((0, 0), (1, 0), (0, 1), (0, 2), (0, 3),
                          (1, 1), (1, 2), (1, 3)):
                stp = halves[h][0]
                for a in range(A):
                    j = a % 4
                    nc.tensor.matmul(
                        stp[32 * j:32 * j + 32, :],
                        w8[:, a, kc, :],
                        ptabs[kc][h][:, a, :],
                        start=(a < 4 and kc == 0),
                        stop=(a >= 4 and kc == KC - 1),
                        tile_position=(0, 32 * j),
                    )
                if (h, kc) in ((0, 3), (1, 3)):
                    stp, cs, ao = halves[h]
                    nc.scalar.activation(cs[:], stp[:], CPY, scale=sc,
                                         accum_out=aos[ao][:] if aos else None)

            if last:
                prow = npool.tile([128, 1], bf16, tag="prow")
                with nc.allow_low_precision(reason="prow bf16"):
                    nc.vector.tensor_tensor(prow[:], aos[0][:], aos[1][:], ADD)
                prow_ev = (prow, t)
                break

            # ---- transposes back to T-layout (kc01 first) --------------
            uTTA = tpA.tile([128, 2, 128], bf16, tag="uTTA")
            uTTB = tpB.tile([128, 2, 128], bf16, tag="uTTB")
            for i in range(2):
                nc.tensor.matmul(uTTA[:, i, :], cs01[:, i * 128:(i + 1) * 128],
                                 identB[:], is_transpose=True,
                                 start=True, stop=True)
            for i in range(2):
                nc.tensor.matmul(uTTB[:, i, :], cs23[:, i * 128:(i + 1) * 128],
                                 identB[:], is_transpose=True,
                                 start=True, stop=True)

            # pending event's ell matmul: PE, after the transposes
            if prow_ev is not None:
                ell = ell_matmul(prow_ev[0])
                pending_ln = (ell, prow_ev[1])
                prow_ev = None

            # ---- reduce col-group strips + next w8 (kc01 first) --------
            # uTT free layout per kc: offset = 32*j + e; sum over j
            uT = upool.tile([128, KC, BC], bf16, tag="uT")
            w8n = w8s[(t + 1) % 2]
            qm8n = qm8t.pop(t + 1)

            def red(uTTx, k0):
                ax = uTTx[:]
                rx = bass.AP(ax.tensor, ax.offset,
                             [list(ax.ap[0]), list(ax.ap[1]),
                              [1, BC], [32, 4]])
                nc.vector.tensor_reduce(uT[:, k0:k0 + 2, :], rx, AX, ADD)

            def w8mul(k0, k1):
                ux = uT[:, k0:k1, :]
                bx = bass.AP(ux.tensor, ux.offset,
                             [list(ux.ap[0]), [0, A], list(ux.ap[1]),
                              list(ux.ap[2])])
                nc.vector.tensor_tensor(w8n[:, :, k0:k1, 0:BC], bx,
                                        qm8n[:, :, k0:k1, :], MUL)

            with nc.allow_low_precision(reason="strip-sum of 4 bf16 partials"):
                red(uTTA, 0)
                w8mul(0, 2)      # kc01 gate the next chain's start
                red(uTTB, 2)
                w8mul(2, 4)

            # ---- event tails, all off the critical path ----------------
            if pending_ln is not None:
                ell, ev_t = pending_ln
                if ev_t < TT - 1:
                    # recip first: it gates next iteration's srec replicate
                    rec = npool.tile([BC, 1], bf16, tag="rec")
                    with nc.allow_low_precision(reason="rec bf16"):
                        nc.vector.reciprocal(rec[:], ell[:])
                    pending_srec = rec
                ln_into_logacc(ell)
                pending_ln = None

            if is_event:
                prow = npool.tile([128, 1], bf16, tag="prow")
                with nc.allow_low_precision(reason="prow bf16"):
                    nc.vector.tensor_tensor(prow[:], aos[0][:], aos[1][:], ADD)
                prow_ev = (prow, t)

        # final event (t = 128): ell + ln only
        if prow_ev is not None:
            ell = ell_matmul(prow_ev[0])
            ln_into_logacc(ell)

        nc.sync.dma_start(out_d[:], logacc[:])

    nc.compile()
    return nc


_NC = None


def _get_nc():
    global _NC
    if _NC is None:
        _NC = _build_nc()
    return _NC


def make_in_maps(regime, seq_o, seq_r, seq_d, seq_a,
                 log_init, log_trans, log_emit_o, log_emit_r, log_emit_d,
                 log_policy):
    import ml_dtypes

    q, msk = _host_q_mask(
        np.asarray(regime), np.asarray(seq_o), np.asarray(seq_r),
        np.asarray(seq_d), np.asarray(seq_a),
        np.asarray(log_emit_o, np.float32), np.asarray(log_emit_r, np.float32),
        np.asarray(log_emit_d, np.float32), np.asarray(log_policy, np.float32),
    )
    q[0] *= np.exp(np.asarray(log_init, np.float32))[None, :]

    P = np.exp(np.asarray(log_trans, np.float32))                    # [A,S,S]
    ptab = np.ascontiguousarray(
        P.reshape(A, KC, 128, S).transpose(2, 1, 0, 3)
    ).astype(ml_dtypes.bfloat16)                                     # [128,KC,A,S]
    identB = np.eye(128, dtype=np.float32).astype(ml_dtypes.bfloat16)
    i4t = np.zeros((128, BC), np.float32)                            # prow -> ell
    i4rep = np.zeros((BC, 128), np.float32)                          # rec -> srec
    for j in range(4):
        i4t[32 * j + np.arange(BC), np.arange(BC)] = 1.0
        i4rep[np.arange(BC), 32 * j + np.arange(BC)] = 1.0

    in_maps = []
    for c in range(NCORES):
        bs = c * BC
        # qm8[t, p, a, kc, e] = q[t, bs+e, kc*128+p] * msk[t, a, bs+e]
        qc = q[:, bs:bs + BC, :].reshape(TT, BC, KC, 128)            # [TT,e,kc,p]
        qcT = qc.transpose(0, 3, 2, 1)                               # [TT,p,kc,e]
        mk = msk[:, :, bs:bs + BC]                                   # [TT,a,e]
        qm8 = (qcT[:, :, None, :, :] * mk[:, None, :, None, :])      # [TT,p,a,kc,e]
        qm8 = np.ascontiguousarray(qm8).astype(ml_dtypes.bfloat16)
        in_maps.append({
            "qm8": qm8, "ptab": ptab, "identB": identB,
            "i4t": i4t.astype(ml_dtypes.bfloat16),
            "i4rep": i4rep.astype(ml_dtypes.bfloat16),
        })
    return in_maps


def kernel(regime, seq_o, seq_r, seq_d, seq_a,
           log_init, log_trans, log_emit_o, log_emit_r, log_emit_d,
           log_policy, _trace=False):
    from concourse.bass_utils import run_bass_kernel_spmd

    nc = _get_nc()
    in_maps = make_in_maps(
        regime, seq_o, seq_r, seq_d, seq_a, log_init, log_trans,
        log_emit_o, log_emit_r, log_emit_d, log_policy,
    )
    res = run_bass_kernel_spmd(nc, in_maps, core_ids=list(range(NCORES)),
                               trace=_trace)
    out = np.concatenate([r["out"].reshape(BC) for r in res.results])
    if _trace:
        kernel._last_results = res
    return out.astype(np.float32)
